# revision 20
# baseline (speedup 1.0000x reference)
"""Causal self-attention (S=2048, B=4, D=768, H=12, Hd=64) on 8 TRN2 cores.

Sharding: core c -> (batch b = c//2, head-group hg = c%2).  Each core computes
full-seq attention for one batch element and 6 of the 12 heads plus the Wo
projection restricted to its heads' columns; the host sums the two head-group
partials per batch.

Kernel design (flash-style, fp16 matmuls / fp32 accumulate+softmax):
  xT [768, 2048] fp16 (host-pretransposed) -> 6 SBUF tiles [128d, 2048s]
  qT/kT [384e, 2048s] = WqT/WkT.T @ xT           (e on partition)
  v_aug [2048t, 6, 65] = xT.T @ WvT + ones col   (t on partition)
  per (head h, 512-query group g), over t-blocks tb <= diag:
    scoresT [128t, 512q] = kT-slice.T @ qT-slice  (K=64, into fp32 PSUM,
      two t-blocks share a [128,2,512] PSUM tile)
    + additive causal mask on diagonal blocks
    P'T = exp(0.125 * scoresT)  (one ACT op per 2 t-blocks, fp16 out)
    out_aug [65, 512q] += v_aug-slice.T @ P'T  (ones column accumulates the
      softmax denominator in row 64)
  deferred normalization: denominators collected via DMA into [8, 512]
  per-e-tile tiles, one bulk reciprocal, gpsimd partition-broadcast, then
  in-place multiply of the unnormalized attnT slices
  y [2048s, 768] fp32 = attnT.T @ WoT
"""

import numpy as np

S = 2048
B = 4
D = 768
HD = 64
H = 6          # heads per core
E = H * HD     # 384
ND = D // 128  # 6
NE = E // 128  # 3
NT = S // 128  # 16
NG = S // 512  # 4
NEG = 1e30

_cached = None


def _build():
    import concourse.mybir as mybir
    import concourse.tile as tile
    from concourse import bacc

    f32 = mybir.dt.float32
    f16 = mybir.dt.float16

    nc = bacc.Bacc("TRN2")

    xT_d = nc.dram_tensor("xT", [D, S], f16, kind="ExternalInput")
    wq_d = nc.dram_tensor("WqT", [D, E], f16, kind="ExternalInput")
    wk_d = nc.dram_tensor("WkT", [D, E], f16, kind="ExternalInput")
    wv_d = nc.dram_tensor("WvT", [D, E], f16, kind="ExternalInput")
    wo_d = nc.dram_tensor("WoT", [E, D], f16, kind="ExternalInput")
    mask_d = nc.dram_tensor("mask", [128, 4, 2, 512], f16, kind="ExternalInput")
    y_d = nc.dram_tensor("y", [S, D], f32, kind="ExternalOutput")

    with tile.TileContext(nc) as tc:
        with (
            tc.tile_pool(name="xt", bufs=6) as xt_pool,
            tc.tile_pool(name="at", bufs=3) as at_pool,
            tc.tile_pool(name="w", bufs=1) as w_pool,
            tc.tile_pool(name="qk", bufs=6) as qk_pool,
            tc.tile_pool(name="vaug", bufs=16) as v_pool,
            tc.tile_pool(name="pt", bufs=4) as pt_pool,
            tc.tile_pool(name="ep", bufs=6) as ep_pool,
            tc.tile_pool(name="y", bufs=2) as y_pool,
            tc.tile_pool(name="pss", bufs=3, space="PSUM") as pss_pool,
            tc.tile_pool(name="pso", bufs=2, space="PSUM") as pso_pool,
        ):
            wq = w_pool.tile([128, ND, E], f16, tag="wq")
            wk = w_pool.tile([128, ND, E], f16, tag="wk")
            wv = w_pool.tile([128, ND, E], f16, tag="wv")
            wo = w_pool.tile([128, NE, D], f16, tag="wo")
            mask = w_pool.tile([128, 4, 2, 512], f16, tag="mask")
            ones = w_pool.tile([128, 1], f16, tag="ones")
            dn0 = w_pool.tile([8, 512], f32, tag="dn0")
            dn1 = w_pool.tile([8, 512], f32, tag="dn1")
            dn2 = w_pool.tile([8, 512], f32, tag="dn2")
            dns = [dn0, dn1, dn2]

            nc.sync.dma_start(wv[:], wv_d.rearrange("(n p) e -> p n e", p=128))
            xT = []
            for d in range(ND):
                t = xt_pool.tile([128, S], f16, tag="xt")
                xT.append(t)
            for ch in range(4):
                for d in range(ND):
                    nc.sync.dma_start(
                        xT[d][:, ch * 512 : (ch + 1) * 512],
                        xT_d[d * 128 : (d + 1) * 128, ch * 512 : (ch + 1) * 512],
                    )
            nc.sync.dma_start(wq[:], wq_d.rearrange("(n p) e -> p n e", p=128))
            nc.sync.dma_start(wk[:], wk_d.rearrange("(n p) e -> p n e", p=128))
            nc.sync.dma_start(wo[:], wo_d.rearrange("(n p) e -> p n e", p=128))
            nc.sync.dma_start(mask[:], mask_d[:])
            nc.vector.memset(ones[:], 1.0)

            # ---- v projection (emitted in t-chunks, interleaved below) ----
            vaug = [None] * NT

            def project_v(t0, t1):
                for t in range(t0, t1):
                    ps = pso_pool.tile([128, E], f32, tag="pso", name="psv")
                    for d in range(ND):
                        nc.tensor.matmul(
                            ps[:],
                            xT[d][:, t * 128 : (t + 1) * 128],
                            wv[:, d, :],
                            start=(d == 0),
                            stop=(d == ND - 1),
                        )
                    va = v_pool.tile([128, H, 65], f16, tag="vaug")
                    nc.vector.tensor_copy(
                        va[:, :, 0:64], ps[:].rearrange("p (h e) -> p h e", e=64)
                    )
                    nc.vector.tensor_copy(
                        va[:, :, 64:65], ones[:, None, :].to_broadcast((128, H, 1))
                    )
                    vaug[t] = va

            qT = [None] * NE
            kT = [None] * NE
            attnT = [None] * NE

            def project(dst_list, et, w_t, nm):
                tl = qk_pool.tile([128, S], f16, tag="qk", name=f"{nm}T{et}")
                for ch in range(4):
                    ps = pso_pool.tile([128, 512], f32, tag="pso", name="psp")
                    for d in range(ND):
                        nc.tensor.matmul(
                            ps[:],
                            w_t[:, d, et * 128 : (et + 1) * 128],
                            xT[d][:, ch * 512 : (ch + 1) * 512],
                            start=(d == 0),
                            stop=(d == ND - 1),
                        )
                    nc.vector.tensor_copy(tl[:, ch * 512 : (ch + 1) * 512], ps[:])
                dst_list[et] = tl

            def attend2(et, g):
                # both heads of e-tile et together: per t-block one 2-bank
                # PSUM tile holds both heads' scoresT; one exp covers both.
                if True:
                    ntb = 4 * g + 4
                    po_out = [
                        pso_pool.tile([65, 512], f32, tag="pso", name=f"po{p}")
                        for p in range(2)
                    ]
                    pts = []
                    emitted_out = 0

                    def emit_scores(tb):
                        j = tb - 4 * g
                        w0 = 128 * j if j >= 1 else 0
                        ps_s = pss_pool.tile([128, 2, 512], f32, tag="pss")
                        for p in range(2):
                            po = p * 64
                            nc.tensor.matmul(
                                ps_s[:, p, w0:512],
                                kT[et][po : po + 64, tb * 128 : (tb + 1) * 128],
                                qT[et][po : po + 64, g * 512 + w0 : (g + 1) * 512],
                                start=True,
                                stop=True,
                            )
                        pt = pt_pool.tile([128, 2, 512], f16, tag="pt")
                        nc.scalar.activation(
                            pt[:, :, w0:512],
                            ps_s[:, :, w0:512],
                            mybir.ActivationFunctionType.Exp,
                            scale=0.125,
                        )
                        if j >= 0:
                            nc.vector.tensor_mul(
                                pt[:, :, w0:512],
                                pt[:, :, w0:512],
                                mask[:, j, :, w0:512],
                            )
                        pts.append((pt, w0))

                    def emit_out(tb):
                        pt, w0 = pts[tb]
                        for p in range(2):
                            h = 2 * et + p
                            nc.tensor.matmul(
                                po_out[p][:, w0:512],
                                vaug[tb][:].rearrange("p h e -> p (h e)")[
                                    :, h * 65 : (h + 1) * 65
                                ],
                                pt[:, p, w0:512],
                                start=(tb == 0),
                                stop=(tb == ntb - 1),
                            )

                    for tb in range(ntb):
                        emit_scores(tb)
                        if tb >= 2:
                            emit_out(emitted_out)
                            emitted_out += 1
                    while emitted_out < ntb:
                        emit_out(emitted_out)
                        emitted_out += 1

                    for p in range(2):
                        po = p * 64
                        nc.vector.tensor_copy(
                            attnT[et][po : po + 64, g * 512 : (g + 1) * 512],
                            po_out[p][0:64, :],
                        )
                        idx = p * NG + g
                        dtmp = ep_pool.tile([1, 512], f32, tag="dtmp")
                        nc.vector.tensor_copy(dtmp[:], po_out[p][64:65, :])
                        nc.sync.dma_start(dns[et][idx : idx + 1, :], dtmp[:])

            project_v(0, NT)
            for et in range(NE):
                project(qT, et, wq, "q")
                project(kT, et, wk, "k")
                attnT[et] = at_pool.tile([128, S], f16, tag="at", name=f"attnT{et}")
                for g in range(NG):
                    attend2(et, g)
                nc.vector.reciprocal(dns[et][0:8, :], dns[et][0:8, :])
                for p in range(2):
                    po = p * 64
                    for g in range(NG):
                        idx = p * NG + g
                        tmp = ep_pool.tile([1, 512], f32, tag="tmp")
                        nc.sync.dma_start(tmp[:], dns[et][idx : idx + 1, :])
                        rb = ep_pool.tile([128, 512], f32, tag="rb")
                        nc.gpsimd.partition_broadcast(rb[:], tmp[:])
                        sl = attnT[et][po : po + 64, g * 512 : (g + 1) * 512]
                        nc.vector.tensor_mul(sl, sl, rb[po : po + 64, :])

            # ---- output projection ----
            for t in range(NT):
                ysb = y_pool.tile([128, D], f32, tag="y")
                for ch in range(2):
                    ps = pso_pool.tile([128, 384], f32, tag="pso", name="psw")
                    for e in range(NE):
                        nc.tensor.matmul(
                            ps[:],
                            attnT[e][:, t * 128 : (t + 1) * 128],
                            wo[:, e, ch * 384 : (ch + 1) * 384],
                            start=(e == 0),
                            stop=(e == NE - 1),
                        )
                    nc.scalar.activation(
                        ysb[:, ch * 384 : (ch + 1) * 384],
                        ps[:],
                        mybir.ActivationFunctionType.Copy,
                    )
                nc.sync.dma_start(y_d[t * 128 : (t + 1) * 128, :], ysb[:])

    nc.compile()
    return nc


def _mask_np():
    tp = np.arange(128)[:, None, None, None]
    j = np.arange(4)[None, :, None, None]
    qf = np.arange(512)[None, None, None, :]
    m = np.where(tp <= qf - 128 * j, 1.0, 0.0).astype(np.float16)
    return np.broadcast_to(m, (128, 4, 2, 512)).copy()


def _in_maps(x, Wq, Wk, Wv, Wo):
    mask = _mask_np()
    maps = []
    for c in range(8):
        b, hg = c // 2, c % 2
        rows = slice(hg * E, (hg + 1) * E)
        maps.append(
            {
                "xT": np.ascontiguousarray(x[:, b, :].T).astype(np.float16),
                "WqT": np.ascontiguousarray(Wq[rows].T).astype(np.float16),
                "WkT": np.ascontiguousarray(Wk[rows].T).astype(np.float16),
                "WvT": np.ascontiguousarray(Wv[rows].T).astype(np.float16),
                "WoT": np.ascontiguousarray(Wo[:, rows].T).astype(np.float16),
                "mask": mask,
            }
        )
    return maps


def get_nc():
    global _cached
    if _cached is None:
        _cached = _build()
    return _cached


def kernel(x, Wq, Wk, Wv, Wo):
    from concourse.bass_utils import run_bass_kernel_spmd

    x = np.asarray(x, dtype=np.float32)
    nc = get_nc()
    res = run_bass_kernel_spmd(
        nc, _in_maps(x, Wq, Wk, Wv, Wo), core_ids=list(range(8))
    )
    out = np.empty((S, B, D), dtype=np.float32)
    for b in range(B):
        out[:, b, :] = res.results[2 * b]["y"] + res.results[2 * b + 1]["y"]
    return out


# revision 21
# speedup vs baseline: 1.0708x; 1.0708x over previous
"""Causal self-attention (S=2048, B=4, D=768, H=12, Hd=64) on 8 TRN2 cores.

Sharding: core c -> (batch b = c//2, head-group hg = c%2).  Each core computes
full-seq attention for one batch element and 6 of the 12 heads plus the Wo
projection restricted to its heads' columns; the host sums the two head-group
partials per batch.

Kernel design (flash-style, fp16 matmuls / fp32 accumulate+softmax):
  xT [768, 2048] fp16 (host-pretransposed) -> 6 SBUF tiles [128d, 2048s]
  qT/kT [384e, 2048s] = WqT/WkT.T @ xT           (e on partition)
  v_aug [2048t, 6, 65] = xT.T @ WvT + ones col   (t on partition)
  per (head h, 512-query group g), over t-blocks tb <= diag:
    scoresT [128t, 512q] = kT-slice.T @ qT-slice  (K=64, into fp32 PSUM,
      two t-blocks share a [128,2,512] PSUM tile)
    + additive causal mask on diagonal blocks
    P'T = exp(0.125 * scoresT)  (one ACT op per 2 t-blocks, fp16 out)
    out_aug [65, 512q] += v_aug-slice.T @ P'T  (ones column accumulates the
      softmax denominator in row 64)
  deferred normalization: denominators collected via DMA into [8, 512]
  per-e-tile tiles, one bulk reciprocal, gpsimd partition-broadcast, then
  in-place multiply of the unnormalized attnT slices
  y [2048s, 768] fp32 = attnT.T @ WoT
"""

import numpy as np

S = 2048
B = 4
D = 768
HD = 64
H = 6          # heads per core
E = H * HD     # 384
ND = D // 128  # 6
NE = E // 128  # 3
NT = S // 128  # 16
NG = S // 512  # 4
NEG = 1e30

_cached = None


def _build():
    import concourse.mybir as mybir
    import concourse.tile as tile
    from concourse import bacc

    f32 = mybir.dt.float32
    f16 = mybir.dt.float16

    nc = bacc.Bacc("TRN2")

    xT_d = nc.dram_tensor("xT", [D, S], f16, kind="ExternalInput")
    wq_d = nc.dram_tensor("WqT", [D, E], f16, kind="ExternalInput")
    wk_d = nc.dram_tensor("WkT", [D, E], f16, kind="ExternalInput")
    wv_d = nc.dram_tensor("WvT", [D, E], f16, kind="ExternalInput")
    wo_d = nc.dram_tensor("WoT", [E, D], f16, kind="ExternalInput")
    mask_d = nc.dram_tensor("mask", [128, 4, 2, 512], f16, kind="ExternalInput")
    y_d = nc.dram_tensor("y", [S, D], f32, kind="ExternalOutput")

    with tile.TileContext(nc) as tc:
        with (
            tc.tile_pool(name="xt", bufs=6) as xt_pool,
            tc.tile_pool(name="at", bufs=3) as at_pool,
            tc.tile_pool(name="w", bufs=1) as w_pool,
            tc.tile_pool(name="qk", bufs=6) as qk_pool,
            tc.tile_pool(name="vaug", bufs=16) as v_pool,
            tc.tile_pool(name="pt", bufs=4) as pt_pool,
            tc.tile_pool(name="ep", bufs=6) as ep_pool,
            tc.tile_pool(name="y", bufs=2) as y_pool,
            tc.tile_pool(name="psa", bufs=1, space="PSUM") as psa_pool,
            tc.tile_pool(name="pss", bufs=2, space="PSUM") as pss_pool,
            tc.tile_pool(name="pso", bufs=3, space="PSUM") as pso_pool,
        ):
            wq = w_pool.tile([128, ND, E], f16, tag="wq")
            wk = w_pool.tile([128, ND, E], f16, tag="wk")
            wv = w_pool.tile([128, ND, E], f16, tag="wv")
            wo = w_pool.tile([128, NE, D], f16, tag="wo")
            mask = w_pool.tile([128, 4, 2, 512], f16, tag="mask")
            ones = w_pool.tile([128, 1], f16, tag="ones")
            dn0 = w_pool.tile([8, 512], f32, tag="dn0")
            dn1 = w_pool.tile([8, 512], f32, tag="dn1")
            dn2 = w_pool.tile([8, 512], f32, tag="dn2")
            dns = [dn0, dn1, dn2]

            nc.sync.dma_start(wv[:], wv_d.rearrange("(n p) e -> p n e", p=128))
            xT = []
            for d in range(ND):
                t = xt_pool.tile([128, S], f16, tag="xt")
                xT.append(t)
            for ch in range(4):
                for d in range(ND):
                    nc.sync.dma_start(
                        xT[d][:, ch * 512 : (ch + 1) * 512],
                        xT_d[d * 128 : (d + 1) * 128, ch * 512 : (ch + 1) * 512],
                    )
            nc.sync.dma_start(wq[:], wq_d.rearrange("(n p) e -> p n e", p=128))
            nc.sync.dma_start(wk[:], wk_d.rearrange("(n p) e -> p n e", p=128))
            nc.sync.dma_start(wo[:], wo_d.rearrange("(n p) e -> p n e", p=128))
            nc.sync.dma_start(mask[:], mask_d[:])
            nc.vector.memset(ones[:], 1.0)

            # ---- v projection (emitted in t-chunks, interleaved below) ----
            vaug = [None] * NT

            def project_v(t0, t1):
                for t in range(t0, t1):
                    pool = psa_pool if t % 2 == 0 else pso_pool
                    ps = pool.tile([128, E], f32, tag=pool.name, name="psv")
                    for d in range(ND):
                        nc.tensor.matmul(
                            ps[:],
                            xT[d][:, t * 128 : (t + 1) * 128],
                            wv[:, d, :],
                            start=(d == 0),
                            stop=(d == ND - 1),
                        )
                    va = v_pool.tile([128, H, 65], f16, tag="vaug")
                    nc.vector.tensor_copy(
                        va[:, :, 0:64], ps[:].rearrange("p (h e) -> p h e", e=64)
                    )
                    nc.vector.tensor_copy(
                        va[:, :, 64:65], ones[:, None, :].to_broadcast((128, H, 1))
                    )
                    vaug[t] = va

            qT = [None] * NE
            kT = [None] * NE
            attnT = [None] * NE

            def project(dst_list, et, w_t, nm):
                tl = qk_pool.tile([128, S], f16, tag="qk", name=f"{nm}T{et}")
                for ch in range(4):
                    pool = psa_pool if ch % 2 == 0 else pso_pool
                    ps = pool.tile([128, 512], f32, tag=pool.name, name="psp")
                    for d in range(ND):
                        nc.tensor.matmul(
                            ps[:],
                            w_t[:, d, et * 128 : (et + 1) * 128],
                            xT[d][:, ch * 512 : (ch + 1) * 512],
                            start=(d == 0),
                            stop=(d == ND - 1),
                        )
                    nc.vector.tensor_copy(tl[:, ch * 512 : (ch + 1) * 512], ps[:])
                dst_list[et] = tl

            def attend2(et, g):
                # both heads of e-tile et together: per t-block one 2-bank
                # PSUM tile holds both heads' scoresT; one exp covers both.
                if True:
                    ntb = 4 * g + 4
                    po_out = [
                        pso_pool.tile([65, 512], f32, tag="pso", name=f"po{p}")
                        for p in range(2)
                    ]
                    pts = []
                    emitted_out = 0

                    def emit_scores(tb):
                        j = tb - 4 * g
                        w0 = 128 * j if j >= 1 else 0
                        ps_s = pss_pool.tile([128, 2, 512], f32, tag="pss")
                        for p in range(2):
                            po = p * 64
                            nc.tensor.matmul(
                                ps_s[:, p, w0:512],
                                kT[et][po : po + 64, tb * 128 : (tb + 1) * 128],
                                qT[et][po : po + 64, g * 512 + w0 : (g + 1) * 512],
                                start=True,
                                stop=True,
                            )
                        pt = pt_pool.tile([128, 2, 512], f16, tag="pt")
                        nc.scalar.activation(
                            pt[:, :, w0:512],
                            ps_s[:, :, w0:512],
                            mybir.ActivationFunctionType.Exp,
                            scale=0.125,
                        )
                        if j >= 0:
                            nc.vector.tensor_mul(
                                pt[:, :, w0:512],
                                pt[:, :, w0:512],
                                mask[:, j, :, w0:512],
                            )
                        pts.append((pt, w0))

                    def emit_out(tb):
                        pt, w0 = pts[tb]
                        for p in range(2):
                            h = 2 * et + p
                            nc.tensor.matmul(
                                po_out[p][:, w0:512],
                                vaug[tb][:].rearrange("p h e -> p (h e)")[
                                    :, h * 65 : (h + 1) * 65
                                ],
                                pt[:, p, w0:512],
                                start=(tb == 0),
                                stop=(tb == ntb - 1),
                            )

                    for tb in range(ntb):
                        emit_scores(tb)
                        if tb >= 2:
                            emit_out(emitted_out)
                            emitted_out += 1
                    while emitted_out < ntb:
                        emit_out(emitted_out)
                        emitted_out += 1

                    for p in range(2):
                        po = p * 64
                        nc.vector.tensor_copy(
                            attnT[et][po : po + 64, g * 512 : (g + 1) * 512],
                            po_out[p][0:64, :],
                        )
                        idx = p * NG + g
                        dtmp = ep_pool.tile([1, 512], f32, tag="dtmp")
                        nc.vector.tensor_copy(dtmp[:], po_out[p][64:65, :])
                        nc.sync.dma_start(dns[et][idx : idx + 1, :], dtmp[:])

            project_v(0, NT)
            for et in range(NE):
                project(qT, et, wq, "q")
                project(kT, et, wk, "k")
                attnT[et] = at_pool.tile([128, S], f16, tag="at", name=f"attnT{et}")
                for g in range(NG):
                    attend2(et, g)
                nc.vector.reciprocal(dns[et][0:8, :], dns[et][0:8, :])
                for p in range(2):
                    po = p * 64
                    for g in range(NG):
                        idx = p * NG + g
                        tmp = ep_pool.tile([1, 512], f32, tag="tmp")
                        nc.sync.dma_start(tmp[:], dns[et][idx : idx + 1, :])
                        rb = ep_pool.tile([128, 512], f32, tag="rb")
                        nc.gpsimd.partition_broadcast(rb[:], tmp[:])
                        sl = attnT[et][po : po + 64, g * 512 : (g + 1) * 512]
                        nc.vector.tensor_mul(sl, sl, rb[po : po + 64, :])

            # ---- output projection ----
            for t in range(NT):
                ysb = y_pool.tile([128, D], f32, tag="y")
                for ch in range(2):
                    pool = psa_pool if (2 * t + ch) % 2 == 0 else pso_pool
                    ps = pool.tile([128, 384], f32, tag=pool.name, name="psw")
                    for e in range(NE):
                        nc.tensor.matmul(
                            ps[:],
                            attnT[e][:, t * 128 : (t + 1) * 128],
                            wo[:, e, ch * 384 : (ch + 1) * 384],
                            start=(e == 0),
                            stop=(e == NE - 1),
                        )
                    nc.scalar.activation(
                        ysb[:, ch * 384 : (ch + 1) * 384],
                        ps[:],
                        mybir.ActivationFunctionType.Copy,
                    )
                nc.sync.dma_start(y_d[t * 128 : (t + 1) * 128, :], ysb[:])

    nc.compile()
    return nc


def _mask_np():
    tp = np.arange(128)[:, None, None, None]
    j = np.arange(4)[None, :, None, None]
    qf = np.arange(512)[None, None, None, :]
    m = np.where(tp <= qf - 128 * j, 1.0, 0.0).astype(np.float16)
    return np.broadcast_to(m, (128, 4, 2, 512)).copy()


def _in_maps(x, Wq, Wk, Wv, Wo):
    mask = _mask_np()
    maps = []
    for c in range(8):
        b, hg = c // 2, c % 2
        rows = slice(hg * E, (hg + 1) * E)
        maps.append(
            {
                "xT": np.ascontiguousarray(x[:, b, :].T).astype(np.float16),
                "WqT": np.ascontiguousarray(Wq[rows].T).astype(np.float16),
                "WkT": np.ascontiguousarray(Wk[rows].T).astype(np.float16),
                "WvT": np.ascontiguousarray(Wv[rows].T).astype(np.float16),
                "WoT": np.ascontiguousarray(Wo[:, rows].T).astype(np.float16),
                "mask": mask,
            }
        )
    return maps


def get_nc():
    global _cached
    if _cached is None:
        _cached = _build()
    return _cached


def kernel(x, Wq, Wk, Wv, Wo):
    from concourse.bass_utils import run_bass_kernel_spmd

    x = np.asarray(x, dtype=np.float32)
    nc = get_nc()
    res = run_bass_kernel_spmd(
        nc, _in_maps(x, Wq, Wk, Wv, Wo), core_ids=list(range(8))
    )
    out = np.empty((S, B, D), dtype=np.float32)
    for b in range(B):
        out[:, b, :] = res.results[2 * b]["y"] + res.results[2 * b + 1]["y"]
    return out
